# revision 12
# baseline (speedup 1.0000x reference)
"""GCN (single GCNConv + Cox head) Trainium2 Bass kernel.

Math (per reference):
    src,dst  += self loops;  deg = indegree(dst);  dinv = deg^-1/2
    norm_e   = dinv[src_e] * dinv[dst_e]
    agg[d]   = sum_e norm_e * x[src_e]          (linearity: aggregate first)
    h        = relu(agg @ W.T + b)
    out      = h @ w_reg.T + b_reg

Distribution: destination-sharded edges over 8 cores (12500 dst nodes each);
no collectives — every core receives its own gather table + edge metadata and
writes its output shard.

Device algorithm per core (per 128-dst block):
  - a per-sub-shard relabeled source table is ordered by first use, so each
    block's first-seen rows form a fixed-size run streamed with ONE
    sequential HWDGE DMA into the "stream" slots [0, R_S);
  - rows already seen in an earlier block of the sub (plus in-block dups)
    are pulled by dma_gather into the "gather" slots [R_S, R_S+R_G), with a
    per-block valid count in a register (trailing -1 idx are not processed,
    so SWDGE descriptor generation only pays for real repeats);
  - the segment-sum one-hot (onehot[slot, j] = norm * (j == dst_rel)) is
    built on host and streamed per block ([128, E_BLK] fp16, one DMA);
  - PE accumulates psum[f, j] += msg[e, f]^T @ onehot[e, j] over the block's
    batches (transposed accumulator); ACT copies psum into accT;
  - phase 2 (interleaved): hT = Wt.T @ accT chunk; ACT relu(+b) psum->sbuf;
    cox row = w_reg.T @ relu_hT (+ b_reg); DMA out.

The relabeled tables exist because dma_gather indices are int16; all feature
data movement (~50 MB/core of x rows + one-hots) happens on device. Either
slot region can absorb the other's overflow (a repeat edge can always be
re-streamed as a duplicate row), so the static SPMD shapes always fit.
"""

import os
import numpy as np

N_CORES = 8
BLK = 128        # dst nodes per block == one-hot window
R_G = 128        # gather slots per block
OVF_PAD = 512    # per-sub overflow row region
IDX_MAX = 32000  # int16 table-index budget
_NQ = 4          # SWDGE queues for dma_gather


class Plan:
    def __init__(self, n_feat, nblk, r_s, nsub, bps, tbl_sub, sub_of_blk, gnp):
        self.F = n_feat
        self.NBLK = nblk
        self.R_S = r_s                  # stream slots per block
        self.E_BLK = r_s + R_G          # total slots per block
        self.NB = self.E_BLK // 128     # batches per block
        self.NSUB = nsub
        self.BPS = bps                  # max blocks per sub
        self.TBL_SUB = tbl_sub          # table rows per sub (incl overflow)
        self.SUB_OF_BLK = sub_of_blk    # block -> sub
        self.KK_OF_BLK = None           # block -> index within its sub
        self.NPAD = nblk * BLK
        self.gnp = gnp
        self.in_maps = []


def make_plan(x, edge_index, W, b, w_reg, b_reg, gnp=np.float16,
              n_cores=N_CORES):
    x = np.asarray(x, dtype=np.float32)
    N, F = x.shape
    ns = N // n_cores
    assert ns * n_cores == N
    nblk = (ns + BLK - 1) // BLK

    src = np.asarray(edge_index[0], dtype=np.int64)
    dst = np.asarray(edge_index[1], dtype=np.int64)
    deg = (np.bincount(dst, minlength=N) + 1).astype(np.float64)
    dinv = 1.0 / np.sqrt(deg)
    norm_real = (dinv[src] * dinv[dst]).astype(np.float32)

    # per-core edge lists
    cores = []
    max_blk_cnt = 0
    for c in range(n_cores):
        lo, hi = c * ns, (c + 1) * ns
        m = (dst >= lo) & (dst < hi)
        s_c = np.concatenate([src[m], np.arange(lo, hi)])
        d_c = np.concatenate([dst[m] - lo, np.arange(ns)])
        n_c = np.concatenate([norm_real[m],
                              (1.0 / deg[lo:hi]).astype(np.float32)])
        blk = d_c >> 7
        rel = (d_c & 127).astype(np.int64)
        order = np.lexsort((s_c, blk))
        cores.append((s_c[order], blk[order], rel[order], n_c[order]))
        max_blk_cnt = max(max_blk_cnt,
                          int(np.bincount(blk, minlength=nblk).max()))
    assert max_blk_cnt <= 1024 + R_G, max_blk_cnt

    # stream-run width: cover typical per-block fresh count; rare overflow
    # spills to the per-sub overflow region.
    r_s = min(-(-max_blk_cnt // 128) * 128, 896)
    bps_cap = (IDX_MAX - OVF_PAD) // r_s
    bps = min(bps_cap, 10) if nblk > 10 else nblk
    nsub = -(-nblk // bps)
    bps = -(-nblk // nsub)  # rebalance
    bps += bps % 2  # even: paired gather calls must not straddle subs
    nsub = -(-nblk // bps)
    sub_of_blk = np.minimum(np.arange(nblk) // bps, nsub - 1)
    kk_of_blk = np.arange(nblk) - np.searchsorted(sub_of_blk, sub_of_blk)
    tbl_sub = bps * r_s + OVF_PAD
    assert tbl_sub <= 32600

    plan = Plan(F, nblk, r_s, nsub, bps, tbl_sub, sub_of_blk, gnp)
    plan.KK_OF_BLK = kk_of_blk
    E_BLK, NB = plan.E_BLK, plan.NB

    consts = {
        "wt": np.ascontiguousarray(np.asarray(W, np.float32).T).astype(gnp),
        "bvec": np.asarray(b, np.float32).reshape(F, 1),
        "wreg": np.ascontiguousarray(
            np.asarray(w_reg, np.float32).T).astype(gnp),
        "breg": np.asarray(b_reg, np.float32).reshape(1, 1),
    }

    for c in range(n_cores):
        s_c, blk_c, rel_c, nrm_c = cores[c]
        bstart = np.searchsorted(blk_c, np.arange(nblk))
        bend = np.searchsorted(blk_c, np.arange(nblk) + 1)

        xg = np.zeros((plan.NSUB * tbl_sub, F), dtype=gnp)
        oh = np.zeros((nblk, 128, NB, 128), dtype=gnp)
        idx_arr = np.zeros((nblk, R_G), dtype=np.int16)
        cnts = np.zeros(nblk, dtype=np.int32)

        for s in range(plan.NSUB):
            seen = {}
            ovf_next = bps * r_s
            sub_base = s * tbl_sub
            for k in np.nonzero(sub_of_blk == s)[0]:
                kk = int(kk_of_blk[k])
                e0, e1 = int(bstart[k]), int(bend[k])
                srcs = s_c[e0:e1]
                rels = rel_c[e0:e1]
                nrms = nrm_c[e0:e1]
                stream = []   # (edge_i, row_src) -> run position
                gather = []   # (edge_i, table_idx)
                run_rows = []
                A = r_s // 128
                for i in range(len(srcs)):
                    sv = int(srcs[i])
                    ti = seen.get(sv)
                    if ti is None and len(run_rows) < r_s:
                        pos = len(run_rows)
                        seen[sv] = kk * r_s + (pos % 128) * A + pos // 128
                        stream.append(i)
                        run_rows.append(sv)
                    elif ti is None:
                        # fresh but run full -> overflow region
                        assert ovf_next < tbl_sub, "overflow region full"
                        seen[sv] = ovf_next
                        gather.append((i, ovf_next))
                        ovf_next += 1
                    else:
                        gather.append((i, ti))
                # too many repeats -> re-stream duplicates
                while len(gather) > R_G:
                    i, ti = gather.pop()
                    assert len(run_rows) < r_s
                    run_rows.append(int(srcs[i]))
                    stream.append(i)
                # fill tables / onehot / idx; run row for slot pos lives at
                # table offset (pos%128)*A + pos//128 so the stream DMA's
                # per-partition lines are contiguous in DRAM
                rows = np.asarray(run_rows, dtype=np.int64)
                if rows.size:
                    pp = np.arange(rows.size)
                    perm = (pp % 128) * (r_s // 128) + pp // 128
                    xg[sub_base + kk * r_s + perm] = x[rows].astype(gnp)
                for pos, i in enumerate(stream):
                    p, j = pos % 128, pos // 128
                    oh[k, p, j, rels[i]] = nrms[i]
                for gi, (i, ti) in enumerate(gather):
                    slot = r_s + gi
                    p, j = slot % 128, slot // 128
                    oh[k, p, j, rels[i]] = nrms[i]
                    idx_arr[k, gi] = ti
                cnts[k] = len(gather)
            # overflow rows for this sub
            if ovf_next > bps * r_s:
                inv = {v: kk for kk, v in seen.items()}
                ov = np.array([inv[t] for t in range(bps * r_s, ovf_next)],
                              dtype=np.int64)
                xg[sub_base + bps * r_s:
                   sub_base + bps * r_s + ov.size] = x[ov].astype(gnp)

        # wrap idx per block: [16, R_G/16] replicated to 128 partitions
        iw = idx_arr.reshape(nblk, R_G // 16, 16).transpose(0, 2, 1)
        iw = np.broadcast_to(iw[:, None], (nblk, 8, 16, R_G // 16))
        idx_wr = np.ascontiguousarray(
            iw.reshape(nblk, 128, R_G // 16).transpose(1, 0, 2)
        ).reshape(128, nblk * (R_G // 16))

        plan.in_maps.append({
            "xg": xg,
            "oh": np.ascontiguousarray(oh).reshape(nblk, 128, E_BLK),
            "idxs": idx_wr,
            **consts,
        })
    return plan


# ---------------------------------------------------------------------------
def build_nc(plan):
    import concourse.bacc as bacc
    import concourse.mybir as mybir
    import concourse.tile as tile

    f32 = mybir.dt.float32
    gdt = mybir.dt.from_np(np.dtype(plan.gnp))
    F, NBLK, NB = plan.F, plan.NBLK, plan.NB
    R_S, E_BLK = plan.R_S, plan.E_BLK
    NPAD, TBL = plan.NPAD, plan.TBL_SUB
    IW = R_G // 16

    nc = bacc.Bacc("TRN2", target_bir_lowering=False, debug=False,
                   num_swdge_queues=_NQ)

    xg = nc.dram_tensor("xg", [plan.NSUB * TBL, F], gdt,
                        kind="ExternalInput").ap()
    oh = nc.dram_tensor("oh", [NBLK, 128, E_BLK], gdt,
                        kind="ExternalInput").ap()
    idxs = nc.dram_tensor("idxs", [128, NBLK * IW], mybir.dt.int16,
                          kind="ExternalInput").ap()
    wt = nc.dram_tensor("wt", [F, F], gdt, kind="ExternalInput").ap()
    bvec = nc.dram_tensor("bvec", [F, 1], f32, kind="ExternalInput").ap()
    wreg = nc.dram_tensor("wreg", [F, 1], gdt, kind="ExternalInput").ap()
    breg = nc.dram_tensor("breg", [1, 1], f32, kind="ExternalInput").ap()
    out = nc.dram_tensor("out", [1, NPAD], f32, kind="ExternalOutput").ap()

    CH = 512

    with tile.TileContext(nc) as tc:
        with (
            tc.tile_pool(name="const", bufs=1) as cpool,
            tc.tile_pool(name="stream", bufs=6) as spool,
            tc.tile_pool(name="gat", bufs=6) as gpool,
            tc.tile_pool(name="ohp", bufs=6) as opool,
            tc.tile_pool(name="ps", bufs=4, space="PSUM") as pspool,
            tc.tile_pool(name="ph2", bufs=2, space="PSUM") as ph2pool,
            tc.tile_pool(name="po", bufs=2, space="PSUM") as popool,
            tc.tile_pool(name="hrelu", bufs=2) as hpool,
        ):
            wt_sb = cpool.tile([F, F], gdt)
            b_sb = cpool.tile([F, 1], f32)
            wreg_sb = cpool.tile([F, 1], gdt)
            breg_sb = cpool.tile([1, 1], f32)
            idx_sb = cpool.tile([128, NBLK * IW], mybir.dt.int16)
            accT = cpool.tile([128, NPAD], gdt)
            out_sb = cpool.tile([1, NPAD], f32)

            for sb, dr in ((wt_sb, wt), (b_sb, bvec), (wreg_sb, wreg),
                           (breg_sb, breg), (idx_sb, idxs)):
                nc.sync.dma_start(out=sb[:], in_=dr[:])

            def phase2(c0, c1):
                cw = c1 - c0
                ph = ph2pool.tile([128, CH], f32)
                hr = hpool.tile([128, CH], gdt)
                po = popool.tile([1, CH], f32)
                nc.tensor.matmul(ph[:, :cw], lhsT=wt_sb[:],
                                 rhs=accT[:, c0:c1], start=True, stop=True)
                nc.scalar.activation(hr[:, :cw], ph[:, :cw],
                                     mybir.ActivationFunctionType.Relu,
                                     bias=b_sb[:, :1])
                nc.tensor.matmul(po[:, :cw], lhsT=wreg_sb[:], rhs=hr[:, :cw],
                                 start=True, stop=True)
                nc.scalar.activation(out_sb[:, c0:c1], po[:, :cw],
                                     mybir.ActivationFunctionType.Identity,
                                     bias=breg_sb[:, :1])

            done_cols = 0
            for k in range(NBLK):
                s = int(plan.SUB_OF_BLK[k])
                kk = int(plan.KK_OF_BLK[k])
                st = spool.tile([128, R_S], gdt, tag="st")
                r0 = s * TBL + kk * R_S
                eng_a = nc.sync if k % 2 == 0 else nc.scalar
                eng_b = nc.scalar if k % 2 == 0 else nc.sync
                eng_a.dma_start(
                    out=st[:].rearrange("p (a f) -> p a f", f=F),
                    in_=xg[r0:r0 + R_S, :].rearrange("(p a) f -> p a f",
                                                     p=128),
                )
                if k % 2 == 0:
                    k2 = min(k + 2, NBLK) - k  # blocks served by this call
                    gt2 = gpool.tile([128, 2 * R_G], gdt, tag="gt")
                    nc.gpsimd.dma_gather(
                        out_ap=gt2[:, :k2 * R_G].rearrange(
                            "p (a f) -> p a f", f=F),
                        in_ap=xg[s * TBL:(s + 1) * TBL, :],
                        idxs_ap=idx_sb[:, k * IW:(k + k2) * IW],
                        num_idxs=k2 * R_G,
                        num_idxs_reg=k2 * R_G,
                        elem_size=F,
                        queue_num=(k // 2) % _NQ,
                    )
                gt = gt2[:, (k % 2) * R_G:(k % 2) * R_G + R_G]
                ot = opool.tile([128, E_BLK], gdt, tag="ot")
                eng_b.dma_start(out=ot[:], in_=oh[k])

                ps = pspool.tile([128, 128], f32)
                for j in range(NB):
                    if j * 128 < R_S:
                        lhsT = st[:, j * 128:(j + 1) * 128]
                    else:
                        g0 = j * 128 - R_S
                        lhsT = gt[:, g0:g0 + 128]  # gt is an AP slice
                    nc.tensor.matmul(ps[:], lhsT=lhsT,
                                     rhs=ot[:, j * 128:(j + 1) * 128],
                                     start=(j == 0), stop=(j == NB - 1))
                nc.scalar.activation(accT[:, k * 128:(k + 1) * 128], ps[:],
                                     mybir.ActivationFunctionType.Copy)
                avail = (k + 1) * 128
                while done_cols + CH <= avail or (k == NBLK - 1
                                                  and done_cols < NPAD):
                    c1 = min(done_cols + CH, NPAD)
                    phase2(done_cols, c1)
                    done_cols = c1

            nc.sync.dma_start(out=out[:], in_=out_sb[:])

    nc.compile()
    return nc


# ---------------------------------------------------------------------------
_CACHE = {}


def _ensure_ntff_hook():
    try:
        from antenv.axon_hooks import get_axon_ntff_profile_hook  # noqa: F401
        return
    except ImportError:
        pass
    import sys
    import types
    import antenv
    mod = types.ModuleType("antenv.axon_hooks")
    mod._hook = None
    mod.set_axon_ntff_profile_hook = lambda h: setattr(mod, "_hook", h)
    mod.get_axon_ntff_profile_hook = lambda: mod._hook
    sys.modules["antenv.axon_hooks"] = mod
    antenv.axon_hooks = mod
    try:
        from trn_agent_boot.trn_boot import _ntff_profile_via_ctypes
        mod._hook = _ntff_profile_via_ctypes("/opt/axon/libaxon_pjrt.so")
    except Exception:
        pass


def _patch_ldw_opt():
    import concourse.bass_utils as bu
    if getattr(bu, "_gcn_ldw_patched", False):
        return
    orig = bu.run_command

    def patched(argv, **kw):
        argv = ["--enable-ldw-opt=true" if a == "--enable-ldw-opt=false"
                else a for a in argv]
        return orig(argv, **kw)

    bu.run_command = patched
    bu._gcn_ldw_patched = True


def _run(plan, nc, trace=False):
    import concourse.bass_utils as bu
    if os.environ.get("GCN_LDWOPT"):
        _patch_ldw_opt()
    if trace:
        _ensure_ntff_hook()
        bu.upload_artifacts = lambda tmpdir: tmpdir  # no egress here
    core_ids = list(range(len(plan.in_maps)))
    res = bu.run_bass_kernel_spmd(nc, plan.in_maps, core_ids, trace=trace)
    return res


def kernel(x, edge_index, W, b, w_reg, b_reg):
    gnp = np.float32 if os.environ.get("GCN_F32") else np.float16
    trace = bool(os.environ.get("GCN_TRACE"))

    plan = make_plan(x, edge_index, W, b, w_reg, b_reg, gnp=gnp)
    key = (str(np.dtype(gnp)), plan.NBLK, plan.R_S, plan.NSUB, plan.TBL_SUB)
    if key not in _CACHE:
        _CACHE[key] = build_nc(plan)
    nc = _CACHE[key]

    res = _run(plan, nc, trace=trace)
    kernel.last_exec_ns = res.exec_time_ns
    kernel.last_profile = res.profile_json

    N = np.asarray(x).shape[0]
    ns = N // len(plan.in_maps)
    shards = [res.results[c]["out"][0, :ns] for c in range(len(plan.in_maps))]
    return np.concatenate(shards).reshape(N, 1).astype(np.float32)


kernel.last_exec_ns = None
kernel.last_profile = None


# revision 15
# speedup vs baseline: 1.0571x; 1.0571x over previous
"""GCN (single GCNConv + Cox head) Trainium2 Bass kernel.

Math (per reference):
    src,dst  += self loops;  deg = indegree(dst);  dinv = deg^-1/2
    norm_e   = dinv[src_e] * dinv[dst_e]
    agg[d]   = sum_e norm_e * x[src_e]          (linearity: aggregate first)
    h        = relu(agg @ W.T + b)
    out      = h @ w_reg.T + b_reg

Distribution: destination-sharded edges over 8 cores (12500 dst nodes each);
no collectives — every core receives its own gather table + edge metadata and
writes its output shard.

Device algorithm per core (per 128-dst block):
  - a per-sub-shard relabeled source table is ordered by first use, so each
    block's first-seen rows form a fixed-size run streamed with ONE
    sequential HWDGE DMA into the "stream" slots [0, R_S);
  - rows already seen in an earlier block of the sub (plus in-block dups)
    are pulled by dma_gather into the "gather" slots [R_S, R_S+R_G), with a
    per-block valid count in a register (trailing -1 idx are not processed,
    so SWDGE descriptor generation only pays for real repeats);
  - the segment-sum one-hot (onehot[slot, j] = norm * (j == dst_rel)) is
    built on host and streamed per block ([128, E_BLK] fp16, one DMA);
  - PE accumulates psum[f, j] += msg[e, f]^T @ onehot[e, j] over the block's
    batches (transposed accumulator); ACT copies psum into accT;
  - phase 2 (interleaved): hT = Wt.T @ accT chunk; ACT relu(+b) psum->sbuf;
    cox row = w_reg.T @ relu_hT (+ b_reg); DMA out.

The relabeled tables exist because dma_gather indices are int16; all feature
data movement (~50 MB/core of x rows + one-hots) happens on device. Either
slot region can absorb the other's overflow (a repeat edge can always be
re-streamed as a duplicate row), so the static SPMD shapes always fit.
"""

import os
import numpy as np

N_CORES = 8
BLK = 128        # dst nodes per block == one-hot window
R_G = 128        # gather slots per block
OVF_PAD = 512    # per-sub overflow row region
IDX_MAX = 32000  # int16 table-index budget
_NQ = 4          # SWDGE queues for dma_gather


class Plan:
    def __init__(self, n_feat, nblk, r_s, nsub, bps, tbl_sub, sub_of_blk, gnp):
        self.GCNT = None                # per-block-pair gather idx counts
        self.F = n_feat
        self.NBLK = nblk
        self.R_S = r_s                  # stream slots per block
        self.E_BLK = r_s + R_G          # total slots per block
        self.NB = self.E_BLK // 128     # batches per block
        self.NSUB = nsub
        self.BPS = bps                  # max blocks per sub
        self.TBL_SUB = tbl_sub          # table rows per sub (incl overflow)
        self.SUB_OF_BLK = sub_of_blk    # block -> sub
        self.KK_OF_BLK = None           # block -> index within its sub
        self.NPAD = nblk * BLK
        self.gnp = gnp
        self.in_maps = []


def make_plan(x, edge_index, W, b, w_reg, b_reg, gnp=np.float16,
              n_cores=N_CORES):
    x = np.asarray(x, dtype=np.float32)
    N, F = x.shape
    ns = N // n_cores
    assert ns * n_cores == N
    nblk = (ns + BLK - 1) // BLK

    src = np.asarray(edge_index[0], dtype=np.int64)
    dst = np.asarray(edge_index[1], dtype=np.int64)
    deg = (np.bincount(dst, minlength=N) + 1).astype(np.float64)
    dinv = 1.0 / np.sqrt(deg)
    norm_real = (dinv[src] * dinv[dst]).astype(np.float32)

    # per-core edge lists
    cores = []
    max_blk_cnt = 0
    for c in range(n_cores):
        lo, hi = c * ns, (c + 1) * ns
        m = (dst >= lo) & (dst < hi)
        s_c = np.concatenate([src[m], np.arange(lo, hi)])
        d_c = np.concatenate([dst[m] - lo, np.arange(ns)])
        n_c = np.concatenate([norm_real[m],
                              (1.0 / deg[lo:hi]).astype(np.float32)])
        blk = d_c >> 7
        rel = (d_c & 127).astype(np.int64)
        order = np.lexsort((s_c, blk))
        cores.append((s_c[order], blk[order], rel[order], n_c[order]))
        max_blk_cnt = max(max_blk_cnt,
                          int(np.bincount(blk, minlength=nblk).max()))
    assert max_blk_cnt <= 1024 + R_G, max_blk_cnt

    # stream-run width: cover typical per-block fresh count; rare overflow
    # spills to the per-sub overflow region.
    r_s = min(-(-max_blk_cnt // 128) * 128, 896)
    bps_cap = (IDX_MAX - OVF_PAD) // r_s
    bps = min(bps_cap, 10) if nblk > 10 else nblk
    nsub = -(-nblk // bps)
    bps = -(-nblk // nsub)  # rebalance
    bps += bps % 2  # even: paired gather calls must not straddle subs
    nsub = -(-nblk // bps)
    sub_of_blk = np.minimum(np.arange(nblk) // bps, nsub - 1)
    kk_of_blk = np.arange(nblk) - np.searchsorted(sub_of_blk, sub_of_blk)
    tbl_sub = bps * r_s + OVF_PAD
    assert tbl_sub <= 32600

    plan = Plan(F, nblk, r_s, nsub, bps, tbl_sub, sub_of_blk, gnp)
    plan.KK_OF_BLK = kk_of_blk
    E_BLK, NB = plan.E_BLK, plan.NB

    consts = {
        "wt": np.ascontiguousarray(np.asarray(W, np.float32).T).astype(gnp),
        "bvec": np.asarray(b, np.float32).reshape(F, 1),
        "wreg": np.ascontiguousarray(
            np.asarray(w_reg, np.float32).T).astype(gnp),
        "breg": np.asarray(b_reg, np.float32).reshape(1, 1),
    }

    all_cnts = []
    for c in range(n_cores):
        s_c, blk_c, rel_c, nrm_c = cores[c]
        bstart = np.searchsorted(blk_c, np.arange(nblk))
        bend = np.searchsorted(blk_c, np.arange(nblk) + 1)

        xg = np.zeros((plan.NSUB * tbl_sub, F), dtype=gnp)
        oh = np.zeros((nblk, 128, NB, 128), dtype=gnp)
        idx_arr = np.zeros((nblk, R_G), dtype=np.int16)
        cnts = np.zeros(nblk, dtype=np.int32)
        all_cnts.append(cnts)

        for s in range(plan.NSUB):
            seen = {}
            ovf_next = bps * r_s
            sub_base = s * tbl_sub
            for k in np.nonzero(sub_of_blk == s)[0]:
                kk = int(kk_of_blk[k])
                e0, e1 = int(bstart[k]), int(bend[k])
                srcs = s_c[e0:e1]
                rels = rel_c[e0:e1]
                nrms = nrm_c[e0:e1]
                stream = []   # (edge_i, row_src) -> run position
                gather = []   # (edge_i, table_idx)
                run_rows = []
                A = r_s // 128
                for i in range(len(srcs)):
                    sv = int(srcs[i])
                    ti = seen.get(sv)
                    if ti is None and len(run_rows) < r_s:
                        pos = len(run_rows)
                        seen[sv] = kk * r_s + (pos % 128) * A + pos // 128
                        stream.append(i)
                        run_rows.append(sv)
                    elif ti is None:
                        # fresh but run full -> overflow region
                        assert ovf_next < tbl_sub, "overflow region full"
                        seen[sv] = ovf_next
                        gather.append((i, ovf_next))
                        ovf_next += 1
                    else:
                        gather.append((i, ti))
                # too many repeats -> re-stream duplicates
                while len(gather) > R_G:
                    i, ti = gather.pop()
                    assert len(run_rows) < r_s
                    run_rows.append(int(srcs[i]))
                    stream.append(i)
                # fill tables / onehot / idx; run row for slot pos lives at
                # table offset (pos%128)*A + pos//128 so the stream DMA's
                # per-partition lines are contiguous in DRAM
                rows = np.asarray(run_rows, dtype=np.int64)
                if rows.size:
                    pp = np.arange(rows.size)
                    perm = (pp % 128) * (r_s // 128) + pp // 128
                    xg[sub_base + kk * r_s + perm] = x[rows].astype(gnp)
                for pos, i in enumerate(stream):
                    p, j = pos % 128, pos // 128
                    oh[k, p, j, rels[i]] = nrms[i]
                for gi, (i, ti) in enumerate(gather):
                    slot = r_s + gi
                    p, j = slot % 128, slot // 128
                    oh[k, p, j, rels[i]] = nrms[i]
                    idx_arr[k, gi] = ti
                cnts[k] = len(gather)
            # overflow rows for this sub
            if ovf_next > bps * r_s:
                inv = {v: kk for kk, v in seen.items()}
                ov = np.array([inv[t] for t in range(bps * r_s, ovf_next)],
                              dtype=np.int64)
                xg[sub_base + bps * r_s:
                   sub_base + bps * r_s + ov.size] = x[ov].astype(gnp)

        # wrap idx per block: [16, R_G/16] replicated to 128 partitions
        iw = idx_arr.reshape(nblk, R_G // 16, 16).transpose(0, 2, 1)
        iw = np.broadcast_to(iw[:, None], (nblk, 8, 16, R_G // 16))
        idx_wr = np.ascontiguousarray(
            iw.reshape(nblk, 128, R_G // 16).transpose(1, 0, 2)
        ).reshape(128, nblk * (R_G // 16))

        plan.in_maps.append({
            "xg": xg,
            "oh": np.ascontiguousarray(oh).reshape(nblk, 128, E_BLK),
            "idxs": idx_wr,
            **consts,
        })
    plan.GCNT = np.max(np.stack(all_cnts), axis=0)
    return plan


# ---------------------------------------------------------------------------
def build_nc(plan):
    import concourse.bacc as bacc
    import concourse.mybir as mybir
    import concourse.tile as tile

    f32 = mybir.dt.float32
    gdt = mybir.dt.from_np(np.dtype(plan.gnp))
    F, NBLK, NB = plan.F, plan.NBLK, plan.NB
    R_S, E_BLK = plan.R_S, plan.E_BLK
    NPAD, TBL = plan.NPAD, plan.TBL_SUB
    IW = R_G // 16

    nc = bacc.Bacc("TRN2", target_bir_lowering=False, debug=False,
                   num_swdge_queues=_NQ)

    xg = nc.dram_tensor("xg", [plan.NSUB * TBL, F], gdt,
                        kind="ExternalInput").ap()
    oh = nc.dram_tensor("oh", [NBLK, 128, E_BLK], gdt,
                        kind="ExternalInput").ap()
    idxs = nc.dram_tensor("idxs", [128, NBLK * IW], mybir.dt.int16,
                          kind="ExternalInput").ap()
    wt = nc.dram_tensor("wt", [F, F], gdt, kind="ExternalInput").ap()
    bvec = nc.dram_tensor("bvec", [F, 1], f32, kind="ExternalInput").ap()
    wreg = nc.dram_tensor("wreg", [F, 1], gdt, kind="ExternalInput").ap()
    breg = nc.dram_tensor("breg", [1, 1], f32, kind="ExternalInput").ap()
    out = nc.dram_tensor("out", [1, NPAD], f32, kind="ExternalOutput").ap()

    CH = 512

    with tile.TileContext(nc) as tc:
        with (
            tc.tile_pool(name="const", bufs=1) as cpool,
            tc.tile_pool(name="stream", bufs=14) as spool,
            tc.tile_pool(name="gat", bufs=14) as gpool,
            tc.tile_pool(name="ohp", bufs=14) as opool,
            tc.tile_pool(name="ps", bufs=6, space="PSUM") as pspool,
            tc.tile_pool(name="ph2", bufs=1, space="PSUM") as ph2pool,
            tc.tile_pool(name="po", bufs=1, space="PSUM") as popool,
            tc.tile_pool(name="hrelu", bufs=2) as hpool,
        ):
            wt_sb = cpool.tile([F, F], gdt)
            b_sb = cpool.tile([F, 1], f32)
            wreg_sb = cpool.tile([F, 1], gdt)
            breg_sb = cpool.tile([1, 1], f32)
            idx_sb = cpool.tile([128, NBLK * IW], mybir.dt.int16)
            accT = cpool.tile([128, NPAD], gdt)
            out_sb = cpool.tile([1, NPAD], f32)

            for sb, dr in ((wt_sb, wt), (b_sb, bvec), (wreg_sb, wreg),
                           (breg_sb, breg), (idx_sb, idxs)):
                nc.sync.dma_start(out=sb[:], in_=dr[:])

            def phase2(c0, c1):
                cw = c1 - c0
                ph = ph2pool.tile([128, CH], f32)
                hr = hpool.tile([128, CH], gdt)
                po = popool.tile([1, CH], f32)
                nc.tensor.matmul(ph[:, :cw], lhsT=wt_sb[:],
                                 rhs=accT[:, c0:c1], start=True, stop=True)
                nc.scalar.activation(hr[:, :cw], ph[:, :cw],
                                     mybir.ActivationFunctionType.Relu,
                                     bias=b_sb[:, :1])
                nc.tensor.matmul(po[:, :cw], lhsT=wreg_sb[:], rhs=hr[:, :cw],
                                 start=True, stop=True)
                nc.scalar.activation(out_sb[:, c0:c1], po[:, :cw],
                                     mybir.ActivationFunctionType.Identity,
                                     bias=breg_sb[:, :1])

            done_cols = 0
            for k in range(NBLK):
                s = int(plan.SUB_OF_BLK[k])
                kk = int(plan.KK_OF_BLK[k])
                st = spool.tile([128, R_S], gdt, tag="st")
                r0 = s * TBL + kk * R_S
                eng_a = nc.sync if k % 2 == 0 else nc.scalar
                eng_b = nc.scalar if k % 2 == 0 else nc.sync
                eng_a.dma_start(
                    out=st[:].rearrange("p (a f) -> p a f", f=F),
                    in_=xg[r0:r0 + R_S, :].rearrange("(p a) f -> p a f",
                                                     p=128),
                )
                gt = gpool.tile([128, R_G], gdt, tag="gt")
                nc.gpsimd.dma_gather(
                    out_ap=gt[:].rearrange("p (a f) -> p a f", f=F),
                    in_ap=xg[s * TBL:(s + 1) * TBL, :],
                    idxs_ap=idx_sb[:, k * IW:(k + 1) * IW],
                    num_idxs=R_G,
                    num_idxs_reg=R_G,
                    elem_size=F,
                    queue_num=k % _NQ,
                )
                ot = opool.tile([128, E_BLK], gdt, tag="ot")
                eng_b.dma_start(out=ot[:], in_=oh[k])

                ps = pspool.tile([128, 128], f32)
                for j in range(NB):
                    if j * 128 < R_S:
                        lhsT = st[:, j * 128:(j + 1) * 128]
                    else:
                        g0 = j * 128 - R_S
                        lhsT = gt[:, g0:g0 + 128]  # gt is an AP slice
                    nc.tensor.matmul(ps[:], lhsT=lhsT,
                                     rhs=ot[:, j * 128:(j + 1) * 128],
                                     start=(j == 0), stop=(j == NB - 1))
                nc.scalar.activation(accT[:, k * 128:(k + 1) * 128], ps[:],
                                     mybir.ActivationFunctionType.Copy)
                avail = (k + 1) * 128
                while done_cols + CH <= avail or (k == NBLK - 1
                                                  and done_cols < NPAD):
                    c1 = min(done_cols + CH, NPAD)
                    phase2(done_cols, c1)
                    done_cols = c1

            nc.sync.dma_start(out=out[:], in_=out_sb[:])

    nc.compile()
    return nc


# ---------------------------------------------------------------------------
_CACHE = {}


def _ensure_ntff_hook():
    try:
        from antenv.axon_hooks import get_axon_ntff_profile_hook  # noqa: F401
        return
    except ImportError:
        pass
    import sys
    import types
    import antenv
    mod = types.ModuleType("antenv.axon_hooks")
    mod._hook = None
    mod.set_axon_ntff_profile_hook = lambda h: setattr(mod, "_hook", h)
    mod.get_axon_ntff_profile_hook = lambda: mod._hook
    sys.modules["antenv.axon_hooks"] = mod
    antenv.axon_hooks = mod
    try:
        from trn_agent_boot.trn_boot import _ntff_profile_via_ctypes
        mod._hook = _ntff_profile_via_ctypes("/opt/axon/libaxon_pjrt.so")
    except Exception:
        pass


def _patch_ldw_opt():
    import concourse.bass_utils as bu
    if getattr(bu, "_gcn_ldw_patched", False):
        return
    orig = bu.run_command

    def patched(argv, **kw):
        argv = ["--enable-ldw-opt=true" if a == "--enable-ldw-opt=false"
                else a for a in argv]
        return orig(argv, **kw)

    bu.run_command = patched
    bu._gcn_ldw_patched = True


def _run(plan, nc, trace=False):
    import concourse.bass_utils as bu
    if os.environ.get("GCN_LDWOPT"):
        _patch_ldw_opt()
    if trace:
        _ensure_ntff_hook()
        bu.upload_artifacts = lambda tmpdir: tmpdir  # no egress here
    core_ids = list(range(len(plan.in_maps)))
    res = bu.run_bass_kernel_spmd(nc, plan.in_maps, core_ids, trace=trace)
    return res


def kernel(x, edge_index, W, b, w_reg, b_reg):
    gnp = np.float32 if os.environ.get("GCN_F32") else np.float16
    trace = bool(os.environ.get("GCN_TRACE"))

    plan = make_plan(x, edge_index, W, b, w_reg, b_reg, gnp=gnp)
    key = (str(np.dtype(gnp)), plan.NBLK, plan.R_S, plan.NSUB, plan.TBL_SUB)
    if key not in _CACHE:
        _CACHE[key] = build_nc(plan)
    nc = _CACHE[key]

    res = _run(plan, nc, trace=trace)
    kernel.last_exec_ns = res.exec_time_ns
    kernel.last_profile = res.profile_json

    N = np.asarray(x).shape[0]
    ns = N // len(plan.in_maps)
    shards = [res.results[c]["out"][0, :ns] for c in range(len(plan.in_maps))]
    return np.concatenate(shards).reshape(N, 1).astype(np.float32)


kernel.last_exec_ns = None
kernel.last_profile = None
